# revision 19
# baseline (speedup 1.0000x reference)
"""Grouped single-step GRU (B=1024, U=8, I=H=512) on 8 trn2 NeuronCores.

Strategy: expert-parallel — core u computes GRU unit u for the whole batch.
Host pre-transposes operands so the device kernel is pure GEMM + fused
epilogue with zero on-chip transposes.

Precision split (rel-err budget is 2e-2; measured 9.7e-3):
  - n- and z-gate matmuls run in bf16: full PE rate, FWL hides LDWEIGHTS.
    (z feeds the output as z*(h-n) directly, so fp8 there blows the error
    budget — verified by bit-exact host simulation.)
  - r-gate matmuls (errors damped by sigmoid slope and tanh) run in
    fp8e4m3 with DoubleRow perf mode: the PE virtualizes to 128x256 so
    each matmul contracts 256 rows -> half the matmul count/time for this
    gate. Weights are host-scaled by 16 (into the fp8 normal range); the
    sigmoid activation un-scales via its scale operand.
  - epilogue keeps PSUM-fed ops in f32 and the gate combine in bf16
    (DVE 2x path); output stored bf16, host upcasts.

Software pipelining: each iteration's z and hn matmul groups (which need
the late-arriving h-side data) are deferred into the next block, so the
DMA stream (ordered by first consumption, first tensors split in half for
early semaphores) feeds the PE without stalls. A burst of junk matmuls at
t=0 warms the HAM clock gate while the first loads stream.
"""

import sys

if "/opt/trn_rl_repo" not in sys.path:
    sys.path.insert(0, "/opt/trn_rl_repo")

from contextlib import ExitStack

import ml_dtypes
import numpy as np

import concourse.tile as tile
from concourse import bacc, mybir
from concourse.bass_utils import run_bass_kernel_spmd

B, U, I, H = 1024, 8, 512, 512
G = 3 * H
NB = 512          # moving-operand width (b-half)
NBH = B // NB     # 2 b-halves
KT = I // 128     # 4 contraction chunks
JT = H // 128     # 4 output-gate partition chunks
N_WARMUP_MM = 10  # bridge the first-load latency; HAM warms during it
W8SCALE = 16.0    # fp8 weight pre-scale (undone by the sigmoid scale)

F32 = mybir.dt.float32
BF16 = mybir.dt.bfloat16
F8 = mybir.dt.float8e4
AF = mybir.ActivationFunctionType
ALU = mybir.AluOpType
DR = mybir.MatmulPerfMode.DoubleRow
BF16NP = ml_dtypes.bfloat16
F8NP = ml_dtypes.float8_e4m3

LAST_EXEC_NS = None
TRACE = False
TRACE_DIR = None

_compiled = None


def _ensure_ntff_hook():
    """Provide antenv.axon_hooks + a ctypes NTFF hook when the image lacks
    them (mirrors trn_agent_boot's degraded-silently path), and keep trace
    artifacts local instead of uploading."""
    import contextlib
    import ctypes
    import types

    from concourse import bass_utils as _bu

    _bu.upload_artifacts = lambda tmpdir: f"local://{tmpdir}"

    try:
        from antenv.axon_hooks import get_axon_ntff_profile_hook  # noqa: F401

        return
    except ImportError:
        pass

    import antenv

    mod = types.ModuleType("antenv.axon_hooks")
    _holder = [None]
    mod.set_axon_ntff_profile_hook = lambda h: _holder.__setitem__(0, h)
    mod.get_axon_ntff_profile_hook = lambda: _holder[0]
    sys.modules["antenv.axon_hooks"] = mod
    antenv.axon_hooks = mod

    lib = ctypes.CDLL("/opt/axon/libaxon_pjrt.so")
    if not hasattr(lib, "axon_start_nrt_profile"):
        return
    lib.axon_start_nrt_profile.argtypes = [
        ctypes.POINTER(ctypes.c_int64),
        ctypes.c_size_t,
    ]
    lib.axon_start_nrt_profile.restype = ctypes.c_int64
    lib.axon_stop_nrt_profile.argtypes = [ctypes.c_char_p]
    lib.axon_stop_nrt_profile.restype = ctypes.c_int64

    @contextlib.contextmanager
    def _hook(output_dir, device_ids):
        import jax

        jax.devices()
        if device_ids:
            ids = (ctypes.c_int64 * len(device_ids))(*device_ids)
            rc = lib.axon_start_nrt_profile(ids, len(device_ids))
        else:
            rc = lib.axon_start_nrt_profile(None, 0)
        if rc != 0:
            raise RuntimeError(f"axon_start_nrt_profile rc={rc}")
        try:
            yield
        finally:
            n = lib.axon_stop_nrt_profile(str(output_dir).encode())
            print(f"ntff profile: {n} file(s) written to {output_dir}")

    mod.set_axon_ntff_profile_hook(_hook)


def _build():
    nc = bacc.Bacc(
        "TRN2",
        target_bir_lowering=False,
        debug=False,
        num_devices=U,
    )
    xT = nc.dram_tensor("xT", [NBH, 128, KT * NB], BF16, kind="ExternalInput").ap()
    hT = nc.dram_tensor("hT", [NBH, 128, KT * NB], BF16, kind="ExternalInput").ap()
    x8d = nc.dram_tensor("x8", [NBH, 128, KT, NB], F8, kind="ExternalInput").ap()
    h8d = nc.dram_tensor("h8", [NBH, 128, KT, NB], F8, kind="ExternalInput").ap()
    # bf16 per-gate weight slabs: [j, p, k*128 + c]
    wnd = nc.dram_tensor("wn", [JT, 128, KT * 128], BF16, kind="ExternalInput").ap()
    und = nc.dram_tensor("un", [JT, 128, KT * 128], BF16, kind="ExternalInput").ap()
    wzd = nc.dram_tensor("wz", [JT, 128, KT * 128], BF16, kind="ExternalInput").ap()
    uzd = nc.dram_tensor("uz", [JT, 128, KT * 128], BF16, kind="ExternalInput").ap()
    # r-gate fp8 slabs: [j, p, slot=(src*4+kk*2+ko), m]
    wr8d = nc.dram_tensor("wr8", [JT, 128, 8, 128], F8, kind="ExternalInput").ap()
    biases = nc.dram_tensor("biases", [128, 16], F32, kind="ExternalInput").ap()
    outT = nc.dram_tensor("outT", [H, B], BF16, kind="ExternalOutput").ap()

    with tile.TileContext(nc) as tc, ExitStack() as ctx:
        wpool = ctx.enter_context(tc.tile_pool(name="w", bufs=1))
        xpool = ctx.enter_context(tc.tile_pool(name="x", bufs=1))
        bpool = ctx.enter_context(tc.tile_pool(name="b", bufs=1))
        # per-gate PSUM pools; 4+2+1+1 tiles of [128,512]f32 = all 8 banks.
        # The single-buf pools work because z/hn groups are deferred one
        # block: tile i is written in block i+1 and read at its end, while
        # tile i+1 is only written in block i+2.
        pp_r = ctx.enter_context(tc.tile_pool(name="ps_r", bufs=4, space="PSUM"))
        pp_xn = ctx.enter_context(tc.tile_pool(name="ps_xn", bufs=2, space="PSUM"))
        pp_z = ctx.enter_context(tc.tile_pool(name="ps_z", bufs=1, space="PSUM"))
        pp_hn = ctx.enter_context(tc.tile_pool(name="ps_hn", bufs=1, space="PSUM"))
        epool = ctx.enter_context(tc.tile_pool(name="work", bufs=2))

        jnk = bpool.tile([128, NB], BF16, tag="jnk")
        nc.vector.memset(jnk[:], 0.0)
        pjnk = pp_xn.tile([128, NB], F32, tag="pxn")
        for _ in range(N_WARMUP_MM):
            nc.tensor.matmul(
                pjnk[:], lhsT=jnk[:, 0:128], rhs=jnk[:], start=True, stop=True
            )

        wg = {}   # (name, j) -> bf16 weight tile
        wr8_s = {}
        x_s = {}
        h_s = {}
        x8_s = {}
        h8_s = {}

        def load_wg(eng, dram, j, nm):
            t = wpool.tile([128, KT * 128], BF16, tag=f"{nm}_{j}")
            eng.dma_start(out=t[:], in_=dram[j])
            wg[nm, j] = t

        def load_wr8(eng, j):
            t = wpool.tile([128, 8, 128], F8, tag=f"wr8_{j}")
            eng.dma_start(out=t[:], in_=wr8d[j])
            wr8_s[j] = t

        def load_x8(bh):
            for d, dram, nm in ((x8_s, x8d, "x8"), (h8_s, h8d, "h8")):
                t = xpool.tile([128, KT, NB], F8, tag=f"{nm}_{bh}")
                nc.sync.dma_start(out=t[:], in_=dram[bh])
                d[bh] = t

        def load_xh(bh, eng, which="xh", split=False):
            for d, dram, nm in ((x_s, xT, "x"), (h_s, hT, "h")):
                if nm not in which:
                    continue
                t = xpool.tile([128, KT * NB], BF16, tag=f"{nm}_{bh}")
                if split:
                    half = KT * NB // 2
                    eng.dma_start(out=t[:, :half], in_=dram[bh][:, :half])
                    eng.dma_start(out=t[:, half:], in_=dram[bh][:, half:])
                else:
                    eng.dma_start(out=t[:], in_=dram[bh])
                d[bh] = t

        bt = bpool.tile([128, 16], F32, tag="bias")
        # pre-loop loads, per-queue, in consumption order: the upfront fp8
        # r-gate sweep needs only wr8_* + x8/h8 (1MB) for its first 4us of
        # matmuls, so the bf16 bulk can trail without stalling the PE.
        load_x8(0)                                # sync: x8_0, h8_0
        load_xh(0, nc.sync, "x", split=True)      # sync: x0a, x0b
        load_xh(0, nc.sync, "h", split=True)      # sync: h0a, h0b
        for j in range(JT):
            load_wr8(nc.scalar, j)
        load_wg(nc.scalar, wnd, 0, "wn")
        nc.scalar.dma_start(out=bt[:], in_=biases[:])
        load_wg(nc.scalar, wzd, 0, "wz")
        load_wg(nc.scalar, uzd, 0, "uz")

        # loads issued at the top of block (bh, j)
        deferred_loads = {
            (0, 0): [
                lambda: load_wg(nc.scalar, und, 0, "un"),
                lambda: load_wg(nc.scalar, wnd, 1, "wn"),
                lambda: load_x8(1),
            ],
            (0, 1): [
                lambda: load_wg(nc.scalar, wzd, 1, "wz"),
                lambda: load_wg(nc.scalar, uzd, 1, "uz"),
                lambda: load_wg(nc.scalar, und, 1, "un"),
                lambda: load_wg(nc.scalar, wnd, 2, "wn"),
                lambda: load_xh(1, nc.sync, "x"),
            ],
            (0, 2): [
                lambda: load_wg(nc.scalar, wzd, 2, "wz"),
                lambda: load_wg(nc.scalar, uzd, 2, "uz"),
                lambda: load_wg(nc.scalar, und, 2, "un"),
                lambda: load_wg(nc.scalar, wnd, 3, "wn"),
                lambda: load_xh(1, nc.sync, "h"),
            ],
            (0, 3): [
                lambda: load_wg(nc.scalar, wzd, 3, "wz"),
                lambda: load_wg(nc.scalar, uzd, 3, "uz"),
                lambda: load_wg(nc.scalar, und, 3, "un"),
            ],
        }

        def mm_group(pt, ops):
            for i, (w, r, pm) in enumerate(ops):
                nc.tensor.matmul(
                    pt[:],
                    lhsT=w,
                    rhs=r,
                    start=(i == 0),
                    stop=(i == len(ops) - 1),
                    perf_mode=pm,
                )

        def bf_ops(nm, bh, j, m_s):
            return [
                (wg[nm, j][:, k * 128 : (k + 1) * 128],
                 m_s[:, k * NB : (k + 1) * NB], None)
                for k in range(KT)
            ]

        def z_ops(bh, j):
            return bf_ops("wz", bh, j, x_s[bh]) + bf_ops("uz", bh, j, h_s[bh])

        def r_ops(bh, j):
            ops = []
            for src, m8 in ((0, x8_s[bh]), (1, h8_s[bh])):
                for kk in range(2):
                    s0 = src * 4 + kk * 2
                    ops.append(
                        (wr8_s[j][:, s0 : s0 + 2, :],
                         m8[:, 2 * kk : 2 * kk + 2, :], DR)
                    )
            return ops

        state = {}

        def make_epilogue(bh, j, pr, pz, pxn, phn):
            r_t = epool.tile([128, NB], F32, tag="r")
            z_t = epool.tile([128, NB], BF16, tag="z")
            t_t = epool.tile([128, NB], F32, tag="t")
            s_t = epool.tile([128, NB], F32, tag="s")
            n_t = epool.tile([128, NB], BF16, tag="n")
            d_t = epool.tile([128, NB], BF16, tag="d")
            e_t = epool.tile([128, NB], BF16, tag="e")
            o_t = epool.tile([128, NB], BF16, tag="o")

            def epilogue(c0, c1, z_late=False, store_eng=None):
                cs = slice(c0, c1)
                h_j = h_s[bh][:, j * NB : (j + 1) * NB]
                nc.scalar.activation(
                    r_t[:, cs], pr[:, cs], AF.Sigmoid,
                    bias=bt[:, j : j + 1], scale=1.0 / W8SCALE,
                )

                def act_z():
                    nc.scalar.activation(
                        z_t[:, cs], pz[:, cs], AF.Sigmoid,
                        bias=bt[:, 4 + j : 5 + j],
                    )

                if not z_late:
                    act_z()
                # t = (hn + b_hn) * r
                nc.vector.scalar_tensor_tensor(
                    t_t[:, cs], phn[:, cs], bt[:, 12 + j : 13 + j], r_t[:, cs],
                    op0=ALU.add, op1=ALU.mult,
                )
                nc.vector.tensor_tensor(
                    s_t[:, cs], t_t[:, cs], pxn[:, cs], op=ALU.add
                )
                nc.scalar.activation(
                    n_t[:, cs], s_t[:, cs], AF.Tanh, bias=bt[:, 8 + j : 9 + j]
                )
                if z_late:
                    act_z()
                # out = n + z * (h - n), all-bf16 on the DVE 2x path
                nc.vector.tensor_tensor(
                    d_t[:, cs], h_j[:, cs], n_t[:, cs], op=ALU.subtract
                )
                nc.vector.tensor_tensor(
                    e_t[:, cs], z_t[:, cs], d_t[:, cs], op=ALU.mult
                )
                nc.vector.tensor_tensor(
                    o_t[:, cs], n_t[:, cs], e_t[:, cs], op=ALU.add
                )
                (store_eng or nc.sync).dma_start(
                    out=outT[
                        j * 128 : (j + 1) * 128, bh * NB + c0 : bh * NB + c1
                    ],
                    in_=o_t[:, cs],
                )

            return epilogue

        def flush_prev(prev):
            if prev is None:
                return
            pbh, pj = prev
            ps = state[prev]
            mm_group(ps["pz"], z_ops(pbh, pj))
            mm_group(ps["phn"], bf_ops("un", pbh, pj, h_s[pbh]))
            ps["epilogue"](0, NB)

        prev = None
        prs = {}
        for bh in range(NBH):
            # fp8 r-gate sweep for the whole batch half, then the deferred
            # z/hn/epilogue of the half boundary
            prs[bh, 0] = pp_r.tile([128, NB], F32, tag="pr", name="pr")
            mm_group(prs[bh, 0], r_ops(bh, 0))
            flush_prev(prev)
            prev = None
            for j in range(1, JT):
                prs[bh, j] = pp_r.tile([128, NB], F32, tag="pr", name="pr")
                mm_group(prs[bh, j], r_ops(bh, j))
            for j in range(JT):
                for fn in deferred_loads.get((bh, j), []):
                    fn()
                pxn = pp_xn.tile([128, NB], F32, tag="pxn")
                pz = pp_z.tile([128, NB], F32, tag="pz")
                phn = pp_hn.tile([128, NB], F32, tag="phn")
                mm_group(pxn, bf_ops("wn", bh, j, x_s[bh]))
                flush_prev(prev)
                state[bh, j] = {
                    "pz": pz,
                    "phn": phn,
                    "epilogue": make_epilogue(
                        bh, j, prs[bh, j], pz, pxn, phn
                    ),
                }
                prev = (bh, j)

        # drain block: hn then z last, so only the short z-tail of the
        # final epilogue pokes out; stores spread over both queues
        ps = state[prev]
        bh, j = prev
        mm_group(ps["phn"], bf_ops("un", bh, j, h_s[bh]))
        mm_group(ps["pz"], z_ops(bh, j))
        ps["epilogue"](0, NB // 2, z_late=True, store_eng=nc.scalar)
        ps["epilogue"](NB // 2, NB, z_late=True, store_eng=nc.sync)

    nc.compile()
    return nc


def _get_nc():
    global _compiled
    if _compiled is None:
        _compiled = _build()
    return _compiled


def _prep_in_maps(inputs, hidden, W_ih, W_hh, b_ih, b_hh):
    def pack_xh(a, np_dt):
        # [B, U, I] -> [U, bh, p, k*NB + b]: tile[p, k*NB+b] = a[bh*NB+b, u, k*128+p]
        a = np.asarray(a, dtype=np.float32)
        a5 = a.reshape(NBH, NB, U, KT, 128)  # [bh, b, u, k, p]
        return (
            a5.transpose(2, 0, 4, 3, 1).astype(np_dt).reshape(U, NBH, 128, KT * NB)
        )

    x = pack_xh(inputs, BF16NP)
    h = pack_xh(hidden, BF16NP)
    x8 = pack_xh(inputs, F8NP).reshape(U, NBH, 128, KT, NB)
    h8 = pack_xh(hidden, F8NP).reshape(U, NBH, 128, KT, NB)

    def pack_gate(W, g):
        # gate-g rows -> [U, JT, 128, KT*128]: slab[j, p, k*128+c]
        # = W[g*512 + j*128 + c, k*128 + p]
        wT = (
            np.asarray(W, dtype=np.float32)[:, g * H : (g + 1) * H, :]
            .transpose(0, 2, 1)
        )
        w5 = wT.reshape(U, KT, 128, JT, 128)  # [u, k, p, j, c]
        return (
            w5.transpose(0, 3, 2, 1, 4).astype(BF16NP).reshape(U, JT, 128, KT * 128)
        )

    wn = pack_gate(W_ih, 2)
    un = pack_gate(W_hh, 2)
    wz = pack_gate(W_ih, 1)
    uz = pack_gate(W_hh, 1)

    def pack_r8(W):
        # r rows, fp8, pre-scaled: [u, j, p, kk, ko, m]
        w = np.asarray(W, dtype=np.float32)[:, :H, :] * W8SCALE
        w6 = w.reshape(U, JT, 128, KT // 2, 2, 128)  # [u,j,m,kk,ko,p]
        return w6.transpose(0, 1, 5, 3, 4, 2)  # [u,j,p,kk,ko,m]

    wr8 = (
        np.stack([pack_r8(W_ih), pack_r8(W_hh)], axis=3)  # [u,j,p,src,kk,ko,m]
        .astype(F8NP)
        .reshape(U, JT, 128, 8, 128)
    )

    bi = np.asarray(b_ih, dtype=np.float32)
    bhh = np.asarray(b_hh, dtype=np.float32)
    brz = bi[:, : 2 * H] + bhh[:, : 2 * H]  # r and z biases combine
    b_in = bi[:, 2 * H :]
    b_hn = bhh[:, 2 * H :]
    in_maps = []
    for u in range(U):
        # [128, 16] tile: column cls*4 + j holds bias_cls[j*128 + p]
        bb = np.stack([brz[u, :H], brz[u, H:], b_in[u], b_hn[u]], axis=0)
        bb = bb.reshape(4, 4, 128).transpose(2, 0, 1).reshape(128, 16)
        in_maps.append(
            {
                "xT": x[u],
                "hT": h[u],
                "x8": x8[u],
                "h8": h8[u],
                "wn": wn[u],
                "un": un[u],
                "wz": wz[u],
                "uz": uz[u],
                "wr8": wr8[u],
                "biases": np.ascontiguousarray(bb),
            }
        )
    return in_maps


def kernel(inputs, hidden, W_ih, W_hh, b_ih, b_hh):
    global LAST_EXEC_NS
    nc = _get_nc()
    in_maps = _prep_in_maps(inputs, hidden, W_ih, W_hh, b_ih, b_hh)
    kwargs = {}
    if TRACE:
        _ensure_ntff_hook()
        if TRACE_DIR is not None:
            import os

            os.makedirs(TRACE_DIR, exist_ok=True)
            kwargs["tmpdir"] = TRACE_DIR
    res = run_bass_kernel_spmd(nc, in_maps, list(range(U)), trace=TRACE, **kwargs)
    LAST_EXEC_NS = res.exec_time_ns
    out = np.empty((B, U, H), dtype=np.float32)
    for u in range(U):
        out[:, u, :] = np.asarray(res.results[u]["outT"]).astype(np.float32).T
    return out
